# revision 4
# baseline (speedup 1.0000x reference)
"""Distributed DPR top-k retrieval kernel for Trainium2 (8 NeuronCores).

Strategy (row-sharded docs, replicated queries):
  - Host: L2-normalize queries, transpose to [D, B]; slice doc matrix into 8
    row shards of N/8 docs and transpose each to [D, N/8] so the contraction
    dim (D) lands on SBUF partitions.
  - Device (SPMD, per core): stream doc tiles [D=768, 2048] from HBM;
      * sims:   psum[64, 512] += qT_chunk.T @ doc_chunk   (6 chunks of 128)
      * norms:  square chunks on ACT, ones-matmul partition-reduce -> psum[1,512]
      * scale:  sqrt (ACT) -> reciprocal (DVE) -> ones-matmul broadcast to
                psum[64,512] -> multiply into cosine sims
      * top-k:  per-2048-group hardware max8 + max_index -> candidate pool
    Final: top-16 of the pool (max8, match_replace, max8) + one-hot index
    recovery; outputs [64,16] vals f32 + local idx int32 per core.
  - Host: merge 8x16 candidates per query (value desc, index asc), take k.
"""

import sys

sys.path.insert(0, "/opt/trn_rl_repo")

import numpy as np

from concourse import bacc, mybir, tile
from concourse.bass_utils import run_bass_kernel_spmd

N_CORES = 8
B = 64
D = 768
D_CHUNKS = 6  # 768 / 128
P = 128
N_TOTAL = 500000
N_LOCAL = N_TOTAL // N_CORES  # 62500
TILE_N = 2048  # docs per DMA tile / max8 group
SUB = 512  # psum subtile width (fp32 bank limit)
K_OUT = 16  # candidates shipped per core per query
NEG = -3.0e38

FP32 = mybir.dt.float32
I32 = mybir.dt.int32
U32 = mybir.dt.uint32


def _ceil_div(a, b):
    return (a + b - 1) // b


def build_kernel(n_local=N_LOCAL, tile_n=TILE_N):
    """Build + compile the per-core SPMD program. Same program for all cores."""
    from contextlib import ExitStack

    n_tiles = _ceil_div(n_local, tile_n)
    pool_w = n_tiles * 8
    assert pool_w >= 8

    nc = bacc.Bacc("TRN2", debug=False, target_bir_lowering=False,
                   num_devices=N_CORES)
    qT = nc.dram_tensor("qT", [D, B], FP32, kind="ExternalInput").ap()
    docT = nc.dram_tensor("docT", [D, n_local], FP32, kind="ExternalInput").ap()
    out_vals = nc.dram_tensor("out_vals", [B, K_OUT], FP32,
                              kind="ExternalOutput").ap()
    out_idx = nc.dram_tensor("out_idx", [B, K_OUT], I32,
                             kind="ExternalOutput").ap()

    with tile.TileContext(nc) as tc, ExitStack() as ctx:
        consts = ctx.enter_context(tc.tile_pool(name="consts", bufs=1))
        docs_pool = ctx.enter_context(tc.tile_pool(name="docs", bufs=2))
        sq_pool = ctx.enter_context(tc.tile_pool(name="sq", bufs=3))
        small_pool = ctx.enter_context(tc.tile_pool(name="small", bufs=3))
        raw_pool = ctx.enter_context(tc.tile_pool(name="raw", bufs=3))
        grp_pool = ctx.enter_context(tc.tile_pool(name="grp", bufs=2))
        idx8_pool = ctx.enter_context(tc.tile_pool(name="idx8", bufs=3))
        fin_pool = ctx.enter_context(tc.tile_pool(name="fin", bufs=1))
        psum_acc = ctx.enter_context(tc.tile_pool(name="pacc", bufs=2, space="PSUM"))
        psum_nrm = ctx.enter_context(tc.tile_pool(name="pnrm", bufs=2, space="PSUM"))
        psum_bc = ctx.enter_context(tc.tile_pool(name="pbc", bufs=2, space="PSUM"))

        # --- constants / persistent state ---
        q_sb = consts.tile([P, D_CHUNKS, B], FP32)  # stationary queries
        nc.sync.dma_start(out=q_sb[:], in_=qT.rearrange("(c p) b -> p c b", p=P))
        ones_p = consts.tile([P, 1], FP32)  # norm partition-reduce lhsT
        nc.vector.memset(ones_p[:], 1.0)
        ones_b = consts.tile([1, B], FP32)  # broadcast lhsT
        nc.vector.memset(ones_b[:], 1.0)

        pool_vals = fin_pool.tile([B, pool_w], FP32)
        pool_idx = fin_pool.tile([B, pool_w], FP32)  # doc ids exact in fp32

        # --- streaming phase ---
        for t in range(n_tiles):
            base = t * tile_n
            w_t = min(tile_n, n_local - base)
            dtile = docs_pool.tile([P, D_CHUNKS, tile_n], FP32)
            nc.sync.dma_start(
                out=dtile[:, :, :w_t],
                in_=docT[:, base:base + w_t].rearrange("(c p) n -> p c n", p=P),
            )

            group = grp_pool.tile([B, tile_n], FP32)
            if w_t < tile_n:
                nc.vector.memset(group[:, w_t:], NEG)

            for s in range(_ceil_div(w_t, SUB)):
                w_s = min(SUB, w_t - s * SUB)
                sl = slice(s * SUB, s * SUB + w_s)

                # raw similarity accumulation over 6 contraction chunks
                acc = psum_acc.tile([B, SUB], FP32)
                for c in range(D_CHUNKS):
                    nc.tensor.matmul(
                        acc[:, :w_s], q_sb[:, c, :], dtile[:, c, sl],
                        start=(c == 0), stop=(c == D_CHUNKS - 1),
                    )

                # doc norms: square on ACT, partition-reduce via ones-matmul
                nrm = psum_nrm.tile([1, SUB], FP32)
                for c in range(D_CHUNKS):
                    sq = sq_pool.tile([P, SUB], FP32)
                    nc.scalar.square(sq[:, :w_s], dtile[:, c, sl])
                    nc.tensor.matmul(
                        nrm[:, :w_s], ones_p[:], sq[:, :w_s],
                        start=(c == 0), stop=(c == D_CHUNKS - 1),
                    )
                norm_sb = small_pool.tile([1, SUB], FP32, tag="norm")
                nc.scalar.sqrt(norm_sb[:, :w_s], nrm[:, :w_s])
                inv_sb = small_pool.tile([1, SUB], FP32, tag="inv")
                nc.vector.reciprocal(inv_sb[:, :w_s], norm_sb[:, :w_s])

                # broadcast 1/norm across the 64 query partitions
                bc = psum_bc.tile([B, SUB], FP32)
                nc.tensor.matmul(bc[:, :w_s], ones_b[:], inv_sb[:, :w_s],
                                 start=True, stop=True)

                # evict + scale into the group buffer
                raw = raw_pool.tile([B, SUB], FP32)
                nc.scalar.copy(raw[:, :w_s], acc[:, :w_s])
                nc.vector.tensor_mul(group[:, sl], raw[:, :w_s], bc[:, :w_s])

            # hardware top-8 of this 2048-doc group
            gv = pool_vals[:, t * 8:(t + 1) * 8]
            nc.vector.max(out=gv, in_=group[:])
            gp = idx8_pool.tile([B, 8], U32)
            nc.vector.max_index(out=gp, in_max=gv, in_values=group[:])
            gp_f = idx8_pool.tile([B, 8], FP32, tag="gpf")
            nc.vector.tensor_copy(gp_f[:], gp[:])
            nc.vector.tensor_scalar_add(pool_idx[:, t * 8:(t + 1) * 8],
                                        gp_f[:], float(base))

        # --- final: top-16 of the pool + index recovery ---
        vals_sb = fin_pool.tile([B, K_OUT], FP32)
        fp1 = fin_pool.tile([B, 8], U32)
        fp2 = fin_pool.tile([B, 8], U32)
        nc.vector.max(out=vals_sb[:, 0:8], in_=pool_vals[:])
        nc.vector.max_index(out=fp1[:], in_max=vals_sb[:, 0:8],
                            in_values=pool_vals[:])
        pool_mr = fin_pool.tile([B, pool_w], FP32)
        nc.vector.match_replace(out=pool_mr[:], in_to_replace=vals_sb[:, 0:8],
                                in_values=pool_vals[:], imm_value=NEG)
        nc.vector.max(out=vals_sb[:, 8:16], in_=pool_mr[:])
        nc.vector.max_index(out=fp2[:], in_max=vals_sb[:, 8:16],
                            in_values=pool_mr[:])

        iota = fin_pool.tile([B, pool_w], FP32)
        nc.gpsimd.iota(iota[:], pattern=[[1, pool_w]], base=0,
                       channel_multiplier=0,
                       allow_small_or_imprecise_dtypes=True)
        fp1_f = fin_pool.tile([B, 8], FP32)
        fp2_f = fin_pool.tile([B, 8], FP32)
        nc.vector.tensor_copy(fp1_f[:], fp1[:])
        nc.vector.tensor_copy(fp2_f[:], fp2[:])
        idx_f = fin_pool.tile([B, K_OUT], FP32)
        mask = fin_pool.tile([B, pool_w], FP32)
        sel = fin_pool.tile([B, pool_w], FP32)
        for j in range(K_OUT):
            pos = (fp1_f if j < 8 else fp2_f)[:, j % 8:j % 8 + 1]
            nc.vector.tensor_scalar(mask[:], iota[:], pos, None,
                                    op0=mybir.AluOpType.is_equal)
            nc.vector.tensor_mul(sel[:], mask[:], pool_idx[:])
            nc.vector.reduce_sum(idx_f[:, j:j + 1], sel[:],
                                 axis=mybir.AxisListType.X)
        idx_i = fin_pool.tile([B, K_OUT], I32)
        nc.vector.tensor_copy(idx_i[:], idx_f[:])

        nc.sync.dma_start(out=out_vals, in_=vals_sb[:])
        nc.sync.dma_start(out=out_idx, in_=idx_i[:])

    nc.compile()
    return nc


_CACHED = None


def _get_nc():
    global _CACHED
    if _CACHED is None:
        _CACHED = build_kernel()
    return _CACHED


def kernel(q_embeds, doc_embeds, k_doc):
    k = int(k_doc)
    assert k <= K_OUT
    q = np.asarray(q_embeds, dtype=np.float32)
    docs = np.asarray(doc_embeds, dtype=np.float32)
    assert q.shape == (B, D) and docs.shape == (N_TOTAL, D)

    qn = q / np.linalg.norm(q, axis=1, keepdims=True)
    qT = np.ascontiguousarray(qn.T)
    in_maps = [
        {"qT": qT,
         "docT": np.ascontiguousarray(docs[c * N_LOCAL:(c + 1) * N_LOCAL].T)}
        for c in range(N_CORES)
    ]

    nc = _get_nc()
    res = run_bass_kernel_spmd(nc, in_maps, list(range(N_CORES))).results

    vals = np.stack([res[c]["out_vals"] for c in range(N_CORES)])  # [8,B,16]
    idxs = np.stack([res[c]["out_idx"] for c in range(N_CORES)]).astype(np.int64)
    idxs += (np.arange(N_CORES) * N_LOCAL)[:, None, None]
    mv = vals.transpose(1, 0, 2).reshape(B, -1)
    mi = idxs.transpose(1, 0, 2).reshape(B, -1)

    top_vals = np.empty((B, k), dtype=np.float32)
    top_idx = np.empty((B, k), dtype=np.int32)
    for b in range(B):
        order = np.lexsort((mi[b], -mv[b]))[:k]
        top_vals[b] = mv[b][order]
        top_idx[b] = mi[b][order]
    return top_vals, top_idx


# revision 19
# speedup vs baseline: 1.0297x; 1.0297x over previous
"""Distributed DPR top-k retrieval kernel for Trainium2 (8 NeuronCores).

Strategy (row-sharded docs, replicated queries):
  - Host: L2-normalize queries, transpose to [D, B]; slice doc matrix into 8
    row shards of N/8 docs and transpose each to [D, N/8] so the contraction
    dim (D) lands on SBUF partitions.
  - Device (SPMD, per core): stream doc tiles [D=768, 2048] from HBM;
      * sims:   psum[64, 512] += qT_chunk.T @ doc_chunk   (6 chunks of 128)
      * norms:  square chunks on ACT, ones-matmul partition-reduce -> psum[1,512]
      * scale:  sqrt (ACT) -> reciprocal (DVE) -> ones-matmul broadcast to
                psum[64,512] -> multiply into cosine sims
      * top-k:  per-2048-group hardware max8 + max_index -> candidate pool
    Final: top-16 of the pool (max8, match_replace, max8) + one-hot index
    recovery; outputs [64,16] vals f32 + local idx int32 per core.
  - Host: merge 8x16 candidates per query (value desc, index asc), take k.
"""

import sys

sys.path.insert(0, "/opt/trn_rl_repo")

import numpy as np

from concourse import bacc, mybir, tile
from concourse.bass_utils import run_bass_kernel_spmd

N_CORES = 8
B = 64
D = 768
D_CHUNKS = 6  # 768 / 128
P = 128
N_TOTAL = 500000
N_LOCAL = N_TOTAL // N_CORES  # 62500
TILE_N = 2048  # docs per DMA tile / max8 group
HALF = 1024  # square-instruction width
SUB = 512  # psum subtile width (fp32 bank limit)
K_OUT = 16  # candidates shipped per core per query
NEG = -3.0e38

FP32 = mybir.dt.float32
F32R = mybir.dt.float32r  # 11-mantissa-bit RNE matmul mode, 4x faster than fp32
I32 = mybir.dt.int32
U32 = mybir.dt.uint32


def _ceil_div(a, b):
    return (a + b - 1) // b


def build_kernel(n_local=N_LOCAL, tile_n=TILE_N):
    """Build + compile the per-core SPMD program. Same program for all cores."""
    from contextlib import ExitStack

    n_tiles = _ceil_div(n_local, tile_n)
    pool_w = n_tiles * 8
    assert pool_w >= 8

    nc = bacc.Bacc("TRN2", debug=False, target_bir_lowering=False,
                   num_devices=N_CORES)
    qT = nc.dram_tensor("qT", [D, B], F32R, kind="ExternalInput").ap()
    docT = nc.dram_tensor("docT", [D, n_local], F32R, kind="ExternalInput").ap()
    ones_in = nc.dram_tensor("ones", [P, 1], F32R, kind="ExternalInput").ap()
    out_vals = nc.dram_tensor("out_vals", [B, K_OUT], FP32,
                              kind="ExternalOutput").ap()
    out_idx = nc.dram_tensor("out_idx", [B, K_OUT], I32,
                             kind="ExternalOutput").ap()

    with tile.TileContext(nc) as tc, ExitStack() as ctx:
        consts = ctx.enter_context(tc.tile_pool(name="consts", bufs=1))
        docs_pool = ctx.enter_context(tc.tile_pool(name="docs", bufs=2))
        sq_pool = ctx.enter_context(tc.tile_pool(name="sq", bufs=2))
        small_pool = ctx.enter_context(tc.tile_pool(name="small", bufs=2))
        raw_pool = ctx.enter_context(tc.tile_pool(name="raw", bufs=2))
        grp_pool = ctx.enter_context(tc.tile_pool(name="grp", bufs=2))
        idx8_pool = ctx.enter_context(tc.tile_pool(name="idx8", bufs=2))
        fin_pool = ctx.enter_context(tc.tile_pool(name="fin", bufs=1))
        psum_acc = ctx.enter_context(tc.tile_pool(name="pacc", bufs=2, space="PSUM"))
        psum_nrm = ctx.enter_context(tc.tile_pool(name="pnrm", bufs=2, space="PSUM"))
        psum_bc = ctx.enter_context(tc.tile_pool(name="pbc", bufs=2, space="PSUM"))

        # --- constants / persistent state ---
        q_sb = consts.tile([P, D_CHUNKS, B], F32R)  # stationary queries
        nc.sync.dma_start(out=q_sb[:], in_=qT.rearrange("(c p) b -> p c b", p=P))
        ones_p = consts.tile([P, 1], F32R)  # norm partition-reduce lhsT
        nc.sync.dma_start(out=ones_p[:], in_=ones_in)
        ones_b = consts.tile([1, B], FP32)  # broadcast lhsT
        nc.vector.memset(ones_b[:], 1.0)

        pool_vals = fin_pool.tile([B, pool_w], FP32)
        pool_idx = fin_pool.tile([B, pool_w], FP32)  # doc ids exact in fp32

        # --- streaming phase ---
        for t in range(n_tiles):
            base = t * tile_n
            w_t = min(tile_n, n_local - base)
            dtile = docs_pool.tile([P, D_CHUNKS, tile_n], F32R)
            nc.sync.dma_start(
                out=dtile[:, :, :w_t],
                in_=docT[:, base:base + w_t].rearrange("(c p) n -> p c n", p=P),
            )

            group = grp_pool.tile([B, tile_n], FP32)
            if w_t < tile_n:
                nc.vector.memset(group[:, w_t:], NEG)

            for h in range(_ceil_div(w_t, HALF)):
                w_h = min(HALF, w_t - h * HALF)
                hsl = slice(h * HALF, h * HALF + w_h)
                # wide squares on ACT (fewer instructions than per-subtile)
                sqh = sq_pool.tile([P, D_CHUNKS, HALF], F32R)
                for c in range(D_CHUNKS):
                    nc.scalar.square(sqh[:, c, :w_h],
                                     dtile[:, c, hsl].bitcast(FP32))

                for s2 in range(_ceil_div(w_h, SUB)):
                    w_s = min(SUB, w_h - s2 * SUB)
                    lo = h * HALF + s2 * SUB
                    sl = slice(lo, lo + w_s)
                    ssl = slice(s2 * SUB, s2 * SUB + w_s)

                    # raw similarity accumulation over 6 contraction chunks
                    acc = psum_acc.tile([B, SUB], FP32)
                    for c in range(D_CHUNKS):
                        nc.tensor.matmul(
                            acc[:, :w_s], q_sb[:, c, :], dtile[:, c, sl],
                            start=(c == 0), stop=(c == D_CHUNKS - 1),
                        )

                    # doc norms: partition-reduce squares via ones-matmul
                    nrm = psum_nrm.tile([1, SUB], FP32)
                    for c in range(D_CHUNKS):
                        nc.tensor.matmul(
                            nrm[:, :w_s], ones_p[:], sqh[:, c, ssl],
                            start=(c == 0), stop=(c == D_CHUNKS - 1),
                        )
                    norm_sb = small_pool.tile([1, SUB], FP32, tag="norm")
                    nc.scalar.sqrt(norm_sb[:, :w_s], nrm[:, :w_s])
                    inv_sb = small_pool.tile([1, SUB], FP32, tag="inv")
                    nc.vector.reciprocal(inv_sb[:, :w_s], norm_sb[:, :w_s])

                    # broadcast 1/norm across the 64 query partitions
                    bc = psum_bc.tile([B, SUB], FP32)
                    nc.tensor.matmul(bc[:, :w_s], ones_b[:], inv_sb[:, :w_s],
                                     start=True, stop=True)

                    # evict psum (alternate ACT/DVE to balance), then scale
                    raw = raw_pool.tile([B, SUB], FP32)
                    if (t * 4 + h * 2 + s2) % 2 == 0:
                        nc.scalar.copy(raw[:, :w_s], acc[:, :w_s])
                    else:
                        nc.vector.tensor_copy(raw[:, :w_s], acc[:, :w_s])
                    nc.vector.tensor_mul(group[:, sl], raw[:, :w_s],
                                         bc[:, :w_s])

            # hardware top-8 of this 2048-doc group
            gv = pool_vals[:, t * 8:(t + 1) * 8]
            nc.vector.max(out=gv, in_=group[:])
            gp = idx8_pool.tile([B, 8], U32)
            nc.vector.max_index(out=gp, in_max=gv, in_values=group[:])
            gp_f = idx8_pool.tile([B, 8], FP32, tag="gpf")
            nc.vector.tensor_copy(gp_f[:], gp[:])
            nc.vector.tensor_scalar_add(pool_idx[:, t * 8:(t + 1) * 8],
                                        gp_f[:], float(base))

        # --- final: top-16 of the pool + index recovery ---
        vals_sb = fin_pool.tile([B, K_OUT], FP32)
        fp1 = fin_pool.tile([B, 8], U32)
        fp2 = fin_pool.tile([B, 8], U32)
        nc.vector.max(out=vals_sb[:, 0:8], in_=pool_vals[:])
        nc.vector.max_index(out=fp1[:], in_max=vals_sb[:, 0:8],
                            in_values=pool_vals[:])
        pool_mr = fin_pool.tile([B, pool_w], FP32)
        nc.vector.match_replace(out=pool_mr[:], in_to_replace=vals_sb[:, 0:8],
                                in_values=pool_vals[:], imm_value=NEG)
        nc.vector.max(out=vals_sb[:, 8:16], in_=pool_mr[:])
        nc.vector.max_index(out=fp2[:], in_max=vals_sb[:, 8:16],
                            in_values=pool_mr[:])

        iota = fin_pool.tile([B, pool_w], FP32)
        nc.gpsimd.iota(iota[:], pattern=[[1, pool_w]], base=0,
                       channel_multiplier=0,
                       allow_small_or_imprecise_dtypes=True)
        fp1_f = fin_pool.tile([B, 8], FP32)
        fp2_f = fin_pool.tile([B, 8], FP32)
        nc.vector.tensor_copy(fp1_f[:], fp1[:])
        nc.vector.tensor_copy(fp2_f[:], fp2[:])
        idx_f = fin_pool.tile([B, K_OUT], FP32)
        mask = fin_pool.tile([B, pool_w], FP32)
        sel = fin_pool.tile([B, pool_w], FP32)
        for j in range(K_OUT):
            pos = (fp1_f if j < 8 else fp2_f)[:, j % 8:j % 8 + 1]
            nc.vector.tensor_scalar(mask[:], iota[:], pos, None,
                                    op0=mybir.AluOpType.is_equal)
            nc.vector.tensor_mul(sel[:], mask[:], pool_idx[:])
            nc.vector.reduce_sum(idx_f[:, j:j + 1], sel[:],
                                 axis=mybir.AxisListType.X)
        idx_i = fin_pool.tile([B, K_OUT], I32)
        nc.vector.tensor_copy(idx_i[:], idx_f[:])

        nc.sync.dma_start(out=out_vals, in_=vals_sb[:])
        nc.sync.dma_start(out=out_idx, in_=idx_i[:])

    nc.compile()
    return nc


_CACHED = None


def _get_nc():
    global _CACHED
    if _CACHED is None:
        _CACHED = build_kernel()
    return _CACHED


def kernel(q_embeds, doc_embeds, k_doc):
    k = int(k_doc)
    assert k <= K_OUT
    q = np.asarray(q_embeds, dtype=np.float32)
    docs = np.asarray(doc_embeds, dtype=np.float32)
    assert q.shape == (B, D) and docs.shape == (N_TOTAL, D)

    qn = q / np.linalg.norm(q, axis=1, keepdims=True)
    qT = np.ascontiguousarray(qn.T)
    ones = np.ones((P, 1), dtype=np.float32)
    in_maps = [
        {"qT": qT, "ones": ones,
         "docT": np.ascontiguousarray(docs[c * N_LOCAL:(c + 1) * N_LOCAL].T)}
        for c in range(N_CORES)
    ]

    nc = _get_nc()
    res = run_bass_kernel_spmd(nc, in_maps, list(range(N_CORES))).results

    idxs = np.stack([res[c]["out_idx"] for c in range(N_CORES)]).astype(np.int64)
    idxs += (np.arange(N_CORES) * N_LOCAL)[:, None, None]
    cand = idxs.transpose(1, 0, 2).reshape(B, -1)  # [B, 8*K_OUT]

    # Exact fp32 re-rank of the shipped candidates (device scoring is fp32r,
    # ~2^-12 input rounding; selection margins are far larger than that, but
    # the final ordering near the k-th rank needs full fp32).
    top_vals = np.empty((B, k), dtype=np.float32)
    top_idx = np.empty((B, k), dtype=np.int32)
    for b in range(B):
        ids = np.unique(cand[b])
        cd = docs[ids]
        cdn = cd / np.linalg.norm(cd, axis=1, keepdims=True)
        vals = (cdn @ qn[b]).astype(np.float32)
        order = np.lexsort((ids, -vals))[:k]
        top_vals[b] = vals[order]
        top_idx[b] = ids[order]
    return top_vals, top_idx
